# revision 13
# baseline (speedup 1.0000x reference)
"""DCRNN seq2seq (encoder/decoder DCGRU, K=3 Chebyshev diffusion) on 8 NeuronCores.

Sharding: data-parallel over batch (8 batch elements per core); weights and the
200x200 support replicated; no collectives.

v3 — wavefront encoder + cached diffusions + algebraic decoder feedback:
  - Per-layer diffusion cache XH[l]: each h_l(t) is transposed and diffused
    exactly once; gates of (t,l+1) and (t+1,l) both read the cache (the
    baseline diffused each h twice).
  - Gate matmuls contract 6 K=64 terms (ready/old-state terms first so the PE
    can run while same-step dependencies resolve); ONE fused sigmoid per
    n-chunk computes r and u together into a [128,...] RU tile.
  - Decoder feedback folded algebraically: S_k(proj(h3)) = (S_k h3) @ (Wp Wg_k)
    + s_k (x) (pb Wg_k).  The cached XH[3] (with a constant s12 row 64) feeds
    the layer-0 x-terms directly; the projection itself is pure output work,
    off the critical path.  Decoder t=0 uses an unfolded bias column (the
    baseline's pb-fold was stale at t=0).
  - Encoder cells issued by wavefront diagonal (t+l) in phase waves
    (gates -> rh/diffuse -> cand -> tail/cache-diffuse) so up to 4 independent
    cells keep the tensor engine continuously busy (HAM stays un-throttled).
  - Candidate chunk-pairs col-tiled into ONE psum bank (tile_position
    (0,0)/(0,64)); term-major matmul order reuses LDWEIGHTS across n-chunks.
  - GRU tail b-half + rh-mul b-half on the (otherwise idle) GpSimd engine;
    psum evacuations round-robin Scalar/Vector.

All matmul operands bf16 (fp32 psum accumulate).
"""

import numpy as np
import ml_dtypes

import concourse.bass as bass
import concourse.tile as tile
from concourse import bacc, mybir
from concourse.bass_utils import run_bass_kernel_spmd

BF = ml_dtypes.bfloat16
F32 = np.float32

N = 200
U = 64
L = 4
T = 12
B = 64
NCORES = 8
BL = B // NCORES
M0, M1 = 128, 72
NB = 128  # n width of the 'b' half-tile (xbar transpose needs 128-col tiles)
NCH = [(0, 64), (64, 64), (128, 64), (192, 8)]
WAVE = 4  # max cells in flight per wavefront diagonal

dt = mybir.dt
AF = mybir.ActivationFunctionType

_CACHE = {}


def _build(enc_T=T, dec_T=T, wavefront=True):
    nc = bacc.Bacc()

    d = {}

    def din(name, shape, dtype=dt.bfloat16):
        d[name] = nc.dram_tensor(name, shape, dtype, kind='ExternalInput')

    din('SS0', [M0, 400])
    din('SS1', [M1, 400])
    din('Wp', [U + 1, 200])
    din('s12', [1, BL, 400])
    for p in ('e', 'd'):
        if p == 'e':
            din(p + 'g0x', [200, 3, 128])
            din(p + 'c0x', [200, 3, 64])
        din(p + 'g0h', [64, 3, 128])
        din(p + 'c0h', [64, 3, 64])
        din(p + 'gk0lo', [64, 3, 128])
        din(p + 'gk0hi', [64, 3, 128])
        din(p + 'gk12lo', [64, 3, 2, 128])
        din(p + 'gk12hi', [64, 3, 2, 128])
        din(p + 'cLk0x', [64, 3, 64])
        din(p + 'cLh', [64, 3, 64])
        din(p + 'cLx', [64, 3, 2, 64])
        din(p + 'cLrh', [64, 3, 2, 64])
        din(p + 'bg', [128, 5], dt.float32)
        din(p + 'bc', [128, 5], dt.float32)
    din('dWfg', [64, 128])
    din('dWfc', [64, 64])
    din('dWg12', [65, 2, 128])
    din('dWc12', [65, 2, 64])
    din('xTe', [enc_T, 2, M0, BL, 200])
    din('xfme', [enc_T, 2, M0, BL, 200])
    d['onm'] = nc.dram_tensor('onm', [max(dec_T, 1), 200, BL, 200], dt.float16,
                              kind='ExternalOutput')

    with tile.TileContext(nc) as tc:
        with (
            tc.tile_pool(name='const', bufs=1) as cp,
            tc.tile_pool(name='state', bufs=1) as sp,
            tc.tile_pool(name='work', bufs=2) as wp,
            tc.tile_pool(name='xin', bufs=2) as xp,
            tc.tile_pool(name='dps', bufs=2, space='PSUM') as dps,
            tc.tile_pool(name='gps', bufs=4, space='PSUM') as gps,
            tc.tile_pool(name='cps', bufs=2, space='PSUM') as cps,
        ):
            # ---- load constants / weights ----
            CT = {}
            for name, t_ in d.items():
                if name in ('onm', 'xTe', 'xfme'):
                    continue
                shape = list(t_.shape)
                if shape[0] == 200:  # split node-feature-major weights
                    CT[name + '@a'] = cp.tile([M0] + shape[1:], t_.dtype, name='t' + name + 'a')
                    CT[name + '@b'] = cp.tile([M1] + shape[1:], t_.dtype, name='t' + name + 'b')
                    nc.sync.dma_start(out=CT[name + '@a'], in_=t_[0:M0])
                    nc.sync.dma_start(out=CT[name + '@b'], in_=t_[M0:200])
                else:
                    CT[name] = cp.tile(shape, t_.dtype, name='t' + name)
                    nc.sync.dma_start(out=CT[name], in_=t_[:])
            SS = [CT['SS0'], CT['SS1']]
            Wp = CT['Wp']

            # ---- state (single-buffered; issue order + WAR deps serialize) --
            HA, HB, XH = [], [], []
            for l in range(L):
                r = 65 if l == 3 else 64
                HA.append(sp.tile([r, BL, 128], dt.bfloat16, name=f'HA{l}'))
                HB.append(sp.tile([r, BL, NB], dt.bfloat16, name=f'HB{l}'))
                XH.append(sp.tile([r, BL, 400], dt.bfloat16, name=f'XH{l}'))
                nc.vector.memset(HA[l][:], 0.0)
                nc.vector.memset(HB[l][:], 0.0)
                nc.vector.memset(XH[l][0:64], 0.0)
                if l == 3:
                    nc.vector.memset(HA[l][64:65], 1.0)
                    nc.vector.memset(HB[l][64:65], 1.0)
                    # s12 row for the decoder rank-1 bias fold
                    nc.sync.dma_start(out=XH[l][64:65], in_=d['s12'][:])

            evac_ctr = [0]

            def evac(dst, src):
                # round-robin psum evacuation across Scalar/Vector (3:2 vector)
                i = evac_ctr[0] % 5
                evac_ctr[0] += 1
                if i in (0, 2):
                    nc.scalar.copy(dst, src)
                else:
                    nc.vector.tensor_copy(dst, src)

            memset_ctr = {}

            def fresh_zero_cols(tag, bufs, sub):
                """memset pad columns only for the first `bufs` uses of a tag."""
                n = memset_ctr.get(tag, 0)
                if n < bufs:
                    nc.vector.memset(sub, 0.0)
                    memset_ctr[tag] = n + 1

            def diffuse_pair(lhs0, lhs1):
                ps = dps.tile([M0, 400], dt.float32, name='dp', tag='dps')
                nc.tensor.matmul(ps[:], lhs0, SS[0][:], start=True, stop=False)
                nc.tensor.matmul(ps[:], lhs1, SS[1][:], start=False, stop=True)
                return ps

            def fm(A, Bt, ci, rows=64):
                n0, nw = NCH[ci]
                if ci < 2:
                    return A[0:rows, :, n0:n0 + nw]
                return Bt[0:rows, :, n0 - 128:n0 - 128 + nw]

            def xs(Xt, ci, rows):
                # slice of a full-n (200-col) tile
                n0, nw = NCH[ci]
                return Xt[0:rows, :, n0:n0 + nw]

            def k12(Xt, k, ci, rows=64):
                n0, nw = NCH[ci]
                c0 = 200 * (k - 1) + n0
                return Xt[0:rows, :, c0:c0 + nw]

            CTX = {}

            # ---------------- phase 1: gates ----------------
            def p1(p, l, t):
                RFMa = wp.tile([64, BL, 128], dt.bfloat16, name='RFMa', tag='RFMa', bufs=WAVE)
                RFMb = wp.tile([64, BL, 80], dt.bfloat16, name='RFMb', tag='RFMb', bufs=WAVE)
                UFMa = wp.tile([64, BL, 128], dt.bfloat16, name='UFMa', tag='UFMa', bufs=WAVE)
                UFMb = wp.tile([64, BL, 80], dt.bfloat16, name='UFMb', tag='UFMb', bufs=WAVE)
                bias_col = l
                if l == 0:
                    g0h = CT[p + 'g0h']
                    terms = [
                        (g0h[:, 0, :], lambda ci: fm(HA[0], HB[0], ci)),
                        (g0h[:, 1, :], lambda ci: k12(XH[0], 1, ci)),
                        (g0h[:, 2, :], lambda ci: k12(XH[0], 2, ci)),
                    ]
                    if p == 'e':
                        xfm0, xfm1, Xga, Xgb = CTX['x', t]
                        g0xa, g0xb = CT['eg0x@a'], CT['eg0x@b']
                        terms += [
                            (g0xa[:, 0, :], lambda ci: xs(xfm0, ci, M0)),
                            (g0xb[0:M1, 0, :], lambda ci: xs(xfm1, ci, M1)),
                            (g0xa[:, 1, :], lambda ci: k12(Xga, 1, ci, M0)),
                            (g0xb[0:M1, 1, :], lambda ci: k12(Xgb, 1, ci, M1)),
                            (g0xa[:, 2, :], lambda ci: k12(Xga, 2, ci, M0)),
                            (g0xb[0:M1, 2, :], lambda ci: k12(Xgb, 2, ci, M1)),
                        ]
                    elif t > 0:
                        terms += [
                            (CT['dWfg'][:], lambda ci: fm(HA[3], HB[3], ci)),
                            (CT['dWg12'][:, 0, :], lambda ci: k12(XH[3], 1, ci, 65)),
                            (CT['dWg12'][:, 1, :], lambda ci: k12(XH[3], 2, ci, 65)),
                        ]
                    else:
                        bias_col = 4  # unfolded bias: x == 0 at decoder t=0
                else:
                    gk0lo, gk0hi = CT[p + 'gk0lo'], CT[p + 'gk0hi']
                    gk12lo, gk12hi = CT[p + 'gk12lo'], CT[p + 'gk12hi']
                    terms = [
                        (gk0hi[:, l - 1, :], lambda ci: fm(HA[l], HB[l], ci)),
                        (gk12hi[:, l - 1, 0, :], lambda ci: k12(XH[l], 1, ci)),
                        (gk12hi[:, l - 1, 1, :], lambda ci: k12(XH[l], 2, ci)),
                        (gk0lo[:, l - 1, :], lambda ci: fm(HA[l - 1], HB[l - 1], ci)),
                        (gk12lo[:, l - 1, 0, :], lambda ci: k12(XH[l - 1], 1, ci)),
                        (gk12lo[:, l - 1, 1, :], lambda ci: k12(XH[l - 1], 2, ci)),
                    ]
                pss = [gps.tile([M0, BL, NCH[ci][1]], dt.float32, name='gp', tag='gps')
                       for ci in range(4)]
                nterm = len(terms)
                for j, (w, rhsfn) in enumerate(terms):
                    for ci in range(4):
                        nc.tensor.matmul(pss[ci][:, :, :], w, rhsfn(ci),
                                         start=(j == 0), stop=(j == nterm - 1))
                bg = CT[p + 'bg'][:, bias_col:bias_col + 1]
                for ci, (n0, nw) in enumerate(NCH):
                    if ci < 2:
                        dr = RFMa[:, :, n0:n0 + nw]
                        du = UFMa[:, :, n0:n0 + nw]
                    else:
                        dr = RFMb[:, :, n0 - 128:n0 - 128 + nw]
                        du = UFMb[:, :, n0 - 128:n0 - 128 + nw]
                    nc.scalar.activation(dr, pss[ci][0:64, :, :], AF.Sigmoid,
                                         bias=bg[0:64], scale=1.0)
                    nc.scalar.activation(du, pss[ci][64:128, :, :], AF.Sigmoid,
                                         bias=bg[64:128], scale=1.0)
                CTX['ru', l] = (RFMa, RFMb, UFMa, UFMb)

            # ---------------- phase 2a: r*h, transpose, diffuse ----------------
            def p2a(p, l, t):
                RFMa, RFMb, UFMa, UFMb = CTX['ru', l]
                RHa = wp.tile([64, BL, 128], dt.bfloat16, name='RHa', tag='RHa', bufs=WAVE)
                RHb = wp.tile([64, BL, NB], dt.bfloat16, name='RHb', tag='RHb', bufs=WAVE)
                fresh_zero_cols('RHb', WAVE, RHb[0:64, :, 72:NB])
                nc.vector.tensor_mul(RHa[:], RFMa[:], HA[l][0:64])
                nc.gpsimd.tensor_mul(RHb[0:64, :, 0:72], RFMb[0:64, :, 0:72],
                                     HB[l][0:64, :, 0:72])
                RHT0 = wp.tile([M0, BL, 64], dt.bfloat16, name='RHT0', tag='RHT0')
                RHT1 = wp.tile([NB, BL, 64], dt.bfloat16, name='RHT1', tag='RHT1')
                nc.sync.dma_start_transpose(RHT0[:], RHa[:])
                nc.sync.dma_start_transpose(RHT1[:], RHb[:])
                Xrh = wp.tile([64, BL, 400], dt.bfloat16, name='Xrh', tag='Xrh', bufs=WAVE)
                for b in range(0, BL, 2):
                    ps = diffuse_pair(RHT0[:, b:b + 2, :], RHT1[0:M1, b:b + 2, :])
                    evac(Xrh[0:64, b, :], ps[0:64, :])
                    evac(Xrh[0:64, b + 1, :], ps[64:128, :])
                CTX['rh', l] = (RHa, RHb, Xrh)

            # ---------------- phase 2b: candidate ----------------
            def p2b(p, l, t):
                RHa, RHb, Xrh = CTX['rh', l]
                bias_col = l
                if l == 0:
                    c0h = CT[p + 'c0h']
                    terms = [
                        (c0h[:, 0, :], lambda ci: fm(RHa, RHb, ci)),
                        (c0h[:, 1, :], lambda ci: k12(Xrh, 1, ci)),
                        (c0h[:, 2, :], lambda ci: k12(Xrh, 2, ci)),
                    ]
                    if p == 'e':
                        xfm0, xfm1, Xga, Xgb = CTX['x', t]
                        c0xa, c0xb = CT['ec0x@a'], CT['ec0x@b']
                        terms += [
                            (c0xa[:, 0, :], lambda ci: xs(xfm0, ci, M0)),
                            (c0xb[0:M1, 0, :], lambda ci: xs(xfm1, ci, M1)),
                            (c0xa[:, 1, :], lambda ci: k12(Xga, 1, ci, M0)),
                            (c0xb[0:M1, 1, :], lambda ci: k12(Xgb, 1, ci, M1)),
                            (c0xa[:, 2, :], lambda ci: k12(Xga, 2, ci, M0)),
                            (c0xb[0:M1, 2, :], lambda ci: k12(Xgb, 2, ci, M1)),
                        ]
                    elif t > 0:
                        terms += [
                            (CT['dWfc'][:], lambda ci: fm(HA[3], HB[3], ci)),
                            (CT['dWc12'][:, 0, :], lambda ci: k12(XH[3], 1, ci, 65)),
                            (CT['dWc12'][:, 1, :], lambda ci: k12(XH[3], 2, ci, 65)),
                        ]
                    else:
                        bias_col = 4
                else:
                    cLk0x, cLh = CT[p + 'cLk0x'], CT[p + 'cLh']
                    cLx, cLrh = CT[p + 'cLx'], CT[p + 'cLrh']
                    terms = [
                        (cLk0x[:, l - 1, :], lambda ci: fm(HA[l - 1], HB[l - 1], ci)),
                        (cLx[:, l - 1, 0, :], lambda ci: k12(XH[l - 1], 1, ci)),
                        (cLx[:, l - 1, 1, :], lambda ci: k12(XH[l - 1], 2, ci)),
                        (cLh[:, l - 1, :], lambda ci: fm(RHa, RHb, ci)),
                        (cLrh[:, l - 1, 0, :], lambda ci: k12(Xrh, 1, ci)),
                        (cLrh[:, l - 1, 1, :], lambda ci: k12(Xrh, 2, ci)),
                    ]
                CFMa = wp.tile([64, BL, 128], dt.bfloat16, name='CFMa', tag='CFMa', bufs=WAVE)
                CFMb = wp.tile([64, BL, 80], dt.bfloat16, name='CFMb', tag='CFMb', bufs=WAVE)
                bc = CT[p + 'bc']
                nterm = len(terms)
                pss = [cps.tile([M0, BL, 64], dt.float32, name='cp', tag='cps')
                       for _ in range(2)]
                for j, (w, rhsfn) in enumerate(terms):
                    for pi, (cx, cy) in enumerate(((0, 1), (2, 3))):
                        nwx, nwy = NCH[cx][1], NCH[cy][1]
                        nc.tensor.matmul(pss[pi][0:64, :, 0:nwx], w, rhsfn(cx),
                                         start=(j == 0), stop=(j == nterm - 1),
                                         tile_position=(0, 0))
                        nc.tensor.matmul(pss[pi][64:128, :, 0:nwy], w, rhsfn(cy),
                                         start=(j == 0), stop=(j == nterm - 1),
                                         tile_position=(0, 64))
                for pi, (cx, cy) in enumerate(((0, 1), (2, 3))):
                    for half, ci in ((0, cx), (1, cy)):
                        n0, nw = NCH[ci]
                        if ci < 2:
                            dst = CFMa[0:64, :, n0:n0 + nw]
                        else:
                            dst = CFMb[0:64, :, n0 - 128:n0 - 128 + nw]
                        nc.scalar.activation(
                            dst, pss[pi][64 * half:64 * half + 64, :, 0:nw],
                            AF.Tanh, bias=bc[64 * half:64 * half + 64,
                                             bias_col:bias_col + 1], scale=1.0)
                CTX['cfm', l] = (CFMa, CFMb)

            # ---------------- phase 3: GRU tail + h transpose + cache diffuse --
            def p3(p, l, t):
                RFMa, RFMb, UFMa, UFMb = CTX.pop(('ru', l))
                CFMa, CFMb = CTX.pop(('cfm', l))
                CTX.pop(('rh', l))
                TMPa = wp.tile([64, BL, 128], dt.bfloat16, name='TMPa', tag='TMPa')
                TMPb = wp.tile([64, BL, 72], dt.bfloat16, name='TMPb', tag='TMPb', bufs=1)
                ha = HA[l][0:64]
                nc.vector.tensor_sub(TMPa[:], ha, CFMa[:])
                nc.vector.tensor_mul(TMPa[:], UFMa[:], TMPa[:])
                nc.vector.tensor_add(ha, CFMa[:], TMPa[:])
                hb = HB[l][0:64, :, 0:72]
                cb = CFMb[0:64, :, 0:72]
                nc.gpsimd.tensor_sub(TMPb[:], hb, cb)
                nc.gpsimd.tensor_mul(TMPb[:], UFMb[0:64, :, 0:72], TMPb[:])
                nc.gpsimd.tensor_add(hb, cb, TMPb[:])
                HLT0 = wp.tile([M0, BL, 64], dt.bfloat16, name='HLT0', tag='HLT0')
                HLT1 = wp.tile([NB, BL, 64], dt.bfloat16, name='HLT1', tag='HLT1')
                nc.sync.dma_start_transpose(HLT0[:], HA[l][0:64])
                nc.sync.dma_start_transpose(HLT1[:], HB[l][0:64])
                for b in range(0, BL, 2):
                    ps = diffuse_pair(HLT0[:, b:b + 2, :], HLT1[0:M1, b:b + 2, :])
                    evac(XH[l][0:64, b, :], ps[0:64, :])
                    evac(XH[l][0:64, b + 1, :], ps[64:128, :])

            # ---------------- encoder x: DMA + diffusion ----------------
            def x_load(t):
                x0Ta = xp.tile([M0, BL, 200], dt.bfloat16, name='x0Ta', tag='x0Ta')
                x0Tb = xp.tile([M1, BL, 200], dt.bfloat16, name='x0Tb', tag='x0Tb')
                nc.sync.dma_start(out=x0Ta, in_=d['xTe'][t, 0])
                nc.sync.dma_start(out=x0Tb, in_=d['xTe'][t, 1, 0:M1])
                xfm0 = xp.tile([M0, BL, 200], dt.bfloat16, name='xfm0', tag='xfm0')
                xfm1 = xp.tile([M1, BL, 200], dt.bfloat16, name='xfm1', tag='xfm1')
                nc.sync.dma_start(out=xfm0, in_=d['xfme'][t, 0])
                nc.sync.dma_start(out=xfm1, in_=d['xfme'][t, 1, 0:M1])
                CTX['xload', t] = (x0Ta, x0Tb, xfm0, xfm1)

            def x_diff(t):
                x0Ta, x0Tb, xfm0, xfm1 = CTX.pop(('xload', t))
                Xga = wp.tile([M0, BL, 400], dt.bfloat16, name='Xga', tag='Xga', bufs=1)
                Xgb = wp.tile([M1, BL, 400], dt.bfloat16, name='Xgb', tag='Xgb', bufs=1)
                for b in range(BL):
                    ps = dps.tile([M0, 400], dt.float32, name='dp', tag='dps')
                    nc.tensor.matmul(ps[:], x0Ta[:, b, 0:128], SS[0][:], start=True, stop=False)
                    nc.tensor.matmul(ps[:], x0Tb[0:M1, b, 0:128], SS[1][:], start=False, stop=True)
                    evac(Xga[:, b, :], ps[:, :])
                for b in range(BL):
                    ps = dps.tile([M0, 400], dt.float32, name='dp', tag='dps')
                    nc.tensor.matmul(ps[0:M1, :], x0Ta[:, b, 128:200], SS[0][:], start=True, stop=False)
                    nc.tensor.matmul(ps[0:M1, :], x0Tb[0:M1, b, 128:200], SS[1][:], start=False, stop=True)
                    evac(Xgb[0:M1, b, :], ps[0:M1, :])
                CTX['x', t] = (xfm0, xfm1, Xga, Xgb)

            # ---------------- decoder projection (pure output work) -----------
            def proj(t):
                pT = [wp.tile([M0, BL, 200], dt.float16, name='pT0', tag='pT0', bufs=1),
                      wp.tile([M1, BL, 200], dt.float16, name='pT1', tag='pT1', bufs=1)]
                for mc, M in ((0, M0), (1, M1)):
                    for half in range(4):
                        pps = cps.tile([M0, 2, 200], dt.float32, name='pp', tag='cps')
                        for bb in range(2):
                            b = half * 2 + bb
                            if mc == 0:
                                lhsT = HA[3][0:65, b, 0:M0]
                            else:
                                lhsT = HB[3][0:65, b, 0:M1]
                            nc.tensor.matmul(pps[0:M, bb, :], lhsT, Wp[:],
                                             start=True, stop=True)
                        evac(pT[mc][0:M, half * 2:half * 2 + 2, :], pps[0:M, :, :])
                nc.sync.dma_start(out=d['onm'][t, 0:M0], in_=pT[0][:])
                nc.sync.dma_start(out=d['onm'][t, M0:200], in_=pT[1][0:M1])

            # =================== encoder (wavefront) ===================
            x_load(0)
            if wavefront:
                for dg in range(enc_T + L - 1):
                    cells = [(dg - l, l) for l in range(L) if 0 <= dg - l < enc_T]
                    if dg + 1 < enc_T:
                        x_load(dg + 1)
                    if dg < enc_T:
                        x_diff(dg)
                    for (t, l) in cells:
                        p1('e', l, t)
                    for (t, l) in cells:
                        p2a('e', l, t)
                    for (t, l) in cells:
                        p2b('e', l, t)
                    for (t, l) in cells:
                        p3('e', l, t)
                        if l == 0:
                            CTX.pop(('x', t))
            else:
                for t in range(enc_T):
                    if t + 1 < enc_T:
                        x_load(t + 1)
                    x_diff(t)
                    for l in range(L):
                        p1('e', l, t)
                        p2a('e', l, t)
                        p2b('e', l, t)
                        p3('e', l, t)
                    CTX.pop(('x', t))

            # =================== decoder (sequential) ===================
            for t in range(dec_T):
                if t > 0:
                    proj(t - 1)
                for l in range(L):
                    p1('d', l, t)
                    p2a('d', l, t)
                    p2b('d', l, t)
                    p3('d', l, t)
            proj(dec_T - 1)

    nc.compile()
    return nc


# --------------------------------------------------------------------------
# host-side prep
# --------------------------------------------------------------------------

def _prep_shared(inputs):
    def bfc(x):
        return np.ascontiguousarray(np.asarray(x).astype(BF))

    S = np.asarray(inputs['support'], np.float64)
    S2 = 2.0 * (S @ S) - np.eye(N)
    SS = np.concatenate([S.astype(F32), S2.astype(F32)], axis=1)
    s12 = np.concatenate([S.sum(0), S2.sum(0)]).astype(F32)  # S symmetric
    out = {
        'SS0': bfc(SS[0:M0]),
        'SS1': bfc(SS[M0:200]),
        's12': bfc(np.broadcast_to(s12[None, None, :], (1, BL, 400))),
        'Wp': bfc(np.concatenate(
            [np.asarray(inputs['proj_W'], F32),
             np.asarray(inputs['proj_b'], F32)[None, :]], axis=0)),
    }
    for p, pre in (('e', 'enc_'), ('d', 'dec_')):
        Wg0 = np.asarray(inputs[pre + 'Wg0'], F32).reshape(264, 3, 128)
        Wc0 = np.asarray(inputs[pre + 'Wc0'], F32).reshape(264, 3, 64)
        if p == 'e':
            out[p + 'g0x'] = bfc(Wg0[0:200])
            out[p + 'c0x'] = bfc(Wc0[0:200])
        out[p + 'g0h'] = bfc(Wg0[200:264])
        out[p + 'c0h'] = bfc(Wc0[200:264])
        WgL = np.asarray(inputs[pre + 'Wg'], F32).reshape(3, 128, 3, 128)
        WcL = np.asarray(inputs[pre + 'Wc'], F32).reshape(3, 128, 3, 64)
        out[p + 'gk0lo'] = bfc(WgL[:, 0:64, 0, :].transpose(1, 0, 2))
        out[p + 'gk0hi'] = bfc(WgL[:, 64:128, 0, :].transpose(1, 0, 2))
        out[p + 'gk12lo'] = bfc(WgL[:, 0:64, 1:3, :].transpose(1, 0, 2, 3))
        out[p + 'gk12hi'] = bfc(WgL[:, 64:128, 1:3, :].transpose(1, 0, 2, 3))
        out[p + 'cLk0x'] = bfc(WcL[:, 0:64, 0, :].transpose(1, 0, 2))
        out[p + 'cLh'] = bfc(WcL[:, 64:128, 0, :].transpose(1, 0, 2))
        out[p + 'cLx'] = bfc(WcL[:, 0:64, 1:3, :].transpose(1, 0, 2, 3))
        out[p + 'cLrh'] = bfc(WcL[:, 64:128, 1:3, :].transpose(1, 0, 2, 3))
        bg = np.zeros((128, 5), F32)
        bc = np.zeros((128, 5), F32)
        bg[:, 0] = np.asarray(inputs[pre + 'bg0'], F32)
        bc[0:64, 0] = np.asarray(inputs[pre + 'bc0'], F32)
        bgl = np.asarray(inputs[pre + 'bg'], F32)
        bcl = np.asarray(inputs[pre + 'bc'], F32)
        for l in range(1, 4):
            bg[:, l] = bgl[l - 1]
            bc[0:64, l] = bcl[l - 1]
        bg[:, 4] = bg[:, 0]
        bc[0:64, 4] = bc[0:64, 0]
        if p == 'd':
            pb = np.asarray(inputs['proj_b'], np.float64)
            Wpf = np.asarray(inputs['proj_W'], np.float64)
            bg[:, 0] += (pb @ Wg0[0:200, 0, :].astype(np.float64)).astype(F32)
            bc[0:64, 0] += (pb @ Wc0[0:200, 0, :].astype(np.float64)).astype(F32)
            out['dWfg'] = bfc((Wpf @ Wg0[0:200, 0, :].astype(np.float64)).astype(F32))
            out['dWfc'] = bfc((Wpf @ Wc0[0:200, 0, :].astype(np.float64)).astype(F32))
            dWg12 = np.zeros((65, 2, 128), F32)
            dWc12 = np.zeros((65, 2, 64), F32)
            for k in (1, 2):
                dWg12[0:64, k - 1] = (Wpf @ Wg0[0:200, k, :].astype(np.float64)).astype(F32)
                dWg12[64, k - 1] = (pb @ Wg0[0:200, k, :].astype(np.float64)).astype(F32)
                dWc12[0:64, k - 1] = (Wpf @ Wc0[0:200, k, :].astype(np.float64)).astype(F32)
                dWc12[64, k - 1] = (pb @ Wc0[0:200, k, :].astype(np.float64)).astype(F32)
            out['dWg12'] = bfc(dWg12)
            out['dWc12'] = bfc(dWc12)
        bc[64:128] = bc[0:64]
        out[p + 'bg'] = np.ascontiguousarray(bg)
        out[p + 'bc'] = np.ascontiguousarray(bc)
    return out


def _prep_core_x(x_core, enc_T):
    x = np.asarray(x_core, F32).reshape(BL, -1, N, 200)[:, :enc_T]
    xb = x.astype(BF)
    xTe = np.zeros((enc_T, 2, M0, BL, 200), BF)
    xfme = np.zeros((enc_T, 2, M0, BL, 200), BF)
    xt = xb.transpose(1, 2, 0, 3)  # (T, n, b, f)
    xTe[:, 0, :, :, :] = xt[:, 0:M0]
    xTe[:, 1, 0:M1, :, :] = xt[:, M0:200]
    xf = xb.transpose(1, 3, 0, 2)  # (T, f, b, n)
    xfme[:, 0, :, :, :] = xf[:, 0:M0]
    xfme[:, 1, 0:M1, :, :] = xf[:, M0:200]
    return xTe, xfme


def get_program(enc_T=T, dec_T=T):
    key = (enc_T, dec_T)
    if key not in _CACHE:
        _CACHE[key] = _build(enc_T, dec_T)
    return _CACHE[key]


def make_in_maps(inputs, enc_T=T):
    shared = _prep_shared(inputs)
    x = np.asarray(inputs['inputs'], F32)
    in_maps = []
    for c in range(NCORES):
        xTe, xfme = _prep_core_x(x[c * BL:(c + 1) * BL], enc_T)
        m = dict(shared)
        m['xTe'] = xTe
        m['xfme'] = xfme
        in_maps.append(m)
    return in_maps


def assemble_output(results, dec_T=T):
    out = np.empty((B, dec_T, N * 200), F32)
    for c in range(NCORES):
        onm = results[c]['onm']
        out[c * BL:(c + 1) * BL] = (
            onm[:dec_T].astype(F32).transpose(2, 0, 1, 3).reshape(BL, dec_T, N * 200))
    return out


def kernel(**inputs):
    nc = get_program()
    in_maps = make_in_maps(inputs)
    res = run_bass_kernel_spmd(nc, in_maps, list(range(NCORES))).results
    return assemble_output(res)
